# revision 5
# baseline (speedup 1.0000x reference)
"""Multi-head attention Trainium2 kernel (B=4, S=1024, EMB=1024, 16 heads).

Sharding: 8 cores = 4 batches x 2 head-groups. Core c handles batch c//2 and
heads [8*(c%2), 8*(c%2)+8) -- tensor-parallel over heads within a batch.
Each core computes its Q/K/V projections (512 of 1024 e_out columns), full
attention for its 8 heads, and a partial output projection; the two cores
sharing a batch have their partials summed on the host.

Device layouts (per core):
  QT/KT: [e_out, s] transposed projections as SBUF [128p, 4chunk, 1024s]
         (e_out local = chunk*128 + p; head h at chunk h//2, partitions
         64*(h%2)..+64)
  V:     natural [s, e_out] as SBUF [128p, 8st, 512]
  Scores are computed twice per head: S=[q,k] (softmax rows on partitions,
  normalized weights output) and S^T=[k,q] (feeds P@V with k on partitions).
  Softmax denominators come free from activation(Exp, accum_out=...).
  All matmuls run in float32r (~1.5e-4 rel err, full PE rate at N=512).
"""

import numpy as np

import concourse.bacc as bacc
import concourse.mybir as mybir
import concourse.tile as tile
from concourse.bass_utils import run_bass_kernel_spmd

B, S, EMB, HEADS, HD = 4, 1024, 1024, 16, 64
SCALE = HD**-0.5
NCORES = 8
HPC = HEADS // 2  # heads per core
ESL = HPC * HD  # e_out slice per core (512)
F32 = mybir.dt.float32
F32R = mybir.dt.float32r
EXP = mybir.ActivationFunctionType.Exp
MULT = mybir.AluOpType.mult

_CACHE = {}


def _build():
    if "nc" in _CACHE:
        return _CACHE["nc"]

    nc = bacc.Bacc("TRN2", target_bir_lowering=False, debug=False, num_devices=NCORES)

    xtq = nc.dram_tensor("xtq", [EMB, S], F32R, kind="ExternalInput")
    xtk = nc.dram_tensor("xtk", [EMB, S], F32R, kind="ExternalInput")
    xtv = nc.dram_tensor("xtv", [EMB, S], F32R, kind="ExternalInput")
    wqt = nc.dram_tensor("wqt", [EMB, ESL], F32R, kind="ExternalInput")
    wkt = nc.dram_tensor("wkt", [EMB, ESL], F32R, kind="ExternalInput")
    wvt = nc.dram_tensor("wvt", [EMB, ESL], F32R, kind="ExternalInput")
    wot = nc.dram_tensor("wot", [ESL, EMB], F32R, kind="ExternalInput")
    bq_d = nc.dram_tensor("bq", [128, 4], F32, kind="ExternalInput")
    bk_d = nc.dram_tensor("bk", [128, 4], F32, kind="ExternalInput")
    bo_d = nc.dram_tensor("bo", [128, 8], F32, kind="ExternalInput")
    ones_d = nc.dram_tensor("ones", [1, 128], F32R, kind="ExternalInput")
    wts_d = nc.dram_tensor("wts", [HPC, S, S], F32, kind="ExternalOutput")
    outp_d = nc.dram_tensor("outp", [EMB, S], F32, kind="ExternalOutput")

    with tile.TileContext(nc) as tc, nc.allow_low_precision(
        reason="float32r tiles feed full-rate PE matmuls; accumulation stays fp32"
    ):
        with (
            tc.tile_pool(name="const", bufs=1) as cpool,
            tc.tile_pool(name="qkv", bufs=1) as qkvpool,
            tc.tile_pool(name="wt", bufs=2) as wtpool,
        ):
            bq_sb = cpool.tile([128, 4], F32)
            bk_sb = cpool.tile([128, 4], F32)
            bo_sb = cpool.tile([128, 8], F32)
            ones_sb = cpool.tile([1, 128], F32R)
            nc.sync.dma_start(bq_sb[:], bq_d.ap())
            nc.sync.dma_start(bk_sb[:], bk_d.ap())
            nc.sync.dma_start(bo_sb[:], bo_d.ap())
            nc.sync.dma_start(ones_sb[:], ones_d.ap())

            qt_sb = qkvpool.tile([128, 4, S], F32R)
            kt_sb = qkvpool.tile([128, 4, S], F32R)
            v_sb = qkvpool.tile([128, 8, ESL], F32R)

            # ---- Phase 1: projections ----
            with (
                tc.tile_pool(name="xt", bufs=3) as xtpool,
                tc.tile_pool(name="pjps", bufs=8, space="PSUM") as pjps,
            ):
                for pname, xdram, wdram, bias, dst in (
                    ("q", xtq, wqt, bq_sb, qt_sb),
                    ("k", xtk, wkt, bk_sb, kt_sb),
                    ("v", xtv, wvt, None, v_sb),
                ):
                    w_sb = wtpool.tile([128, 8, ESL], F32R, tag="wt", name=f"w_{pname}")
                    nc.sync.dma_start(
                        w_sb[:], wdram.ap().rearrange("(kt p) n -> p kt n", p=128)
                    )
                    ps = [
                        pjps.tile([128, 512], F32, tag="pjps", name=f"pj_{pname}_{i}")
                        for i in range(8)
                    ]
                    for kt in range(8):
                        x_t = xtpool.tile([128, S], F32R, tag="xt", name=f"x_{pname}_{kt}")
                        nc.sync.dma_start(
                            x_t[:], xdram.ap()[kt * 128 : (kt + 1) * 128, :]
                        )
                        if pname == "v":
                            # V natural [s, e_out]: lhsT = X^T tile cols
                            for st in range(8):
                                nc.tensor.matmul(
                                    ps[st][:],
                                    x_t[:, st * 128 : (st + 1) * 128],
                                    w_sb[:, kt, :],
                                    start=(kt == 0),
                                    stop=(kt == 7),
                                )
                        else:
                            # Q^T/K^T: lhsT = W^T tile cols, rhs = X^T
                            for ch in range(4):
                                for sh in range(2):
                                    nc.tensor.matmul(
                                        ps[ch * 2 + sh][:],
                                        w_sb[:, kt, ch * 128 : (ch + 1) * 128],
                                        x_t[:, sh * 512 : (sh + 1) * 512],
                                        start=(kt == 0),
                                        stop=(kt == 7),
                                    )
                    if pname == "v":
                        for st in range(8):
                            nc.vector.tensor_copy(v_sb[:, st, :], ps[st][:])
                    else:
                        for ch in range(4):
                            for sh in range(2):
                                nc.vector.tensor_scalar_add(
                                    dst[:, ch, sh * 512 : (sh + 1) * 512],
                                    ps[ch * 2 + sh][:],
                                    bias[:, ch : ch + 1],
                                )

            # ---- Phase 2: attention per head ----
            ct_sb = qkvpool.tile([128, 4, S], F32R)
            wo_sb = wtpool.tile([128, 4, EMB], F32R, tag="wt")
            nc.sync.dma_start(
                wo_sb[:], wot.ap().rearrange("(ce p) n -> p ce n", p=128)
            )

            with (
                tc.tile_pool(name="sps", bufs=1, space="PSUM") as spsum,
                tc.tile_pool(name="stps", bufs=1, space="PSUM") as stpsum,
                tc.tile_pool(name="pvps", bufs=2, space="PSUM") as pvpsum,
                tc.tile_pool(name="bcps", bufs=1, space="PSUM") as bcpsum,
                tc.tile_pool(name="enat", bufs=3) as epool,
                tc.tile_pool(name="wsb", bufs=3) as wpool,
                tc.tile_pool(name="et", bufs=3) as etpool,
                tc.tile_pool(name="small", bufs=2) as smallpool,
                tc.tile_pool(name="bc", bufs=2) as bcpool,
            ):
                for h in range(HPC):
                    hb = 64 * (h % 2)
                    hc = h // 2
                    tp = (hb, 0)
                    acc_h = smallpool.tile([128, 8], F32, tag="acc")

                    # S = Q K^T rows-on-partitions; exp + row sums; weights out
                    for qt in range(8):
                        s_ps = spsum.tile([128, 1024], F32, tag="sps")
                        for kh in range(2):
                            nc.tensor.matmul(
                                s_ps[:, kh * 512 : (kh + 1) * 512],
                                qt_sb[hb : hb + 64, hc, qt * 128 : (qt + 1) * 128],
                                kt_sb[hb : hb + 64, hc, kh * 512 : (kh + 1) * 512],
                                start=True,
                                stop=True,
                                tile_position=tp,
                            )
                        e_t = epool.tile([128, 1024], F32, tag="enat")
                        nc.scalar.activation(
                            e_t[:],
                            s_ps[:],
                            EXP,
                            scale=SCALE,
                            accum_out=acc_h[:, qt : qt + 1],
                        )
                        rq = smallpool.tile([128, 1], F32, tag="rq")
                        nc.vector.reciprocal(rq[:], acc_h[:, qt : qt + 1])
                        w_t = wpool.tile([128, 1024], F32, tag="wsb")
                        nc.gpsimd.tensor_scalar_mul(w_t[:], e_t[:], rq[:])
                        nc.sync.dma_start(
                            wts_d.ap()[h, qt * 128 : (qt + 1) * 128, :], w_t[:]
                        )

                    # reciprocal row-sums broadcast tile [128, q] for PV scaling
                    sumsf = smallpool.tile([1, 1024], F32, tag="sumsf")
                    for qt in range(8):
                        nc.sync.dma_start(
                            sumsf[0:1, qt * 128 : (qt + 1) * 128],
                            acc_h[:, qt : qt + 1],
                        )
                    recipf = smallpool.tile([1, 1024], F32R, tag="recipf")
                    nc.vector.reciprocal(recipf[:], sumsf[:])
                    bc_sb = bcpool.tile([128, 1024], F32R, tag="bc")
                    for qh in range(2):
                        bc_ps = bcpsum.tile([128, 512], F32, tag="bcps")
                        nc.tensor.matmul(
                            bc_ps[:],
                            ones_sb[0:1, :],
                            recipf[0:1, qh * 512 : (qh + 1) * 512],
                            start=True,
                            stop=True,
                        )
                        nc.vector.tensor_copy(
                            bc_sb[:, qh * 512 : (qh + 1) * 512], bc_ps[:]
                        )

                    # S^T (k on partitions) -> exp -> P@V accumulation
                    pv0 = pvpsum.tile([64, 512], F32, tag="pv")
                    pv1 = pvpsum.tile([64, 512], F32, tag="pv")
                    for kt in range(8):
                        st_ps = stpsum.tile([128, 1024], F32, tag="stps")
                        for qh in range(2):
                            nc.tensor.matmul(
                                st_ps[:, qh * 512 : (qh + 1) * 512],
                                kt_sb[hb : hb + 64, hc, kt * 128 : (kt + 1) * 128],
                                qt_sb[hb : hb + 64, hc, qh * 512 : (qh + 1) * 512],
                                start=True,
                                stop=True,
                                tile_position=tp,
                            )
                        et_t = etpool.tile([128, 1024], F32R, tag="et")
                        nc.scalar.activation(et_t[:], st_ps[:], EXP, scale=SCALE)
                        for qh, pv in ((0, pv0), (1, pv1)):
                            nc.tensor.matmul(
                                pv[:],
                                v_sb[:, kt, h * 64 : (h + 1) * 64],
                                et_t[:, qh * 512 : (qh + 1) * 512],
                                start=(kt == 0),
                                stop=(kt == 7),
                            )
                    # matmul dst must start at partition 0; odd heads hop to
                    # partitions 64-127 of ct_sb via a small SBUF->SBUF DMA
                    if h % 2 == 0:
                        for qh, pv in ((0, pv0), (1, pv1)):
                            nc.vector.tensor_tensor(
                                ct_sb[0:64, hc, qh * 512 : (qh + 1) * 512],
                                pv[:],
                                bc_sb[0:64, qh * 512 : (qh + 1) * 512],
                                op=MULT,
                            )
                    else:
                        ct_tmp = bcpool.tile([64, 1024], F32R, tag="cttmp")
                        for qh, pv in ((0, pv0), (1, pv1)):
                            nc.vector.tensor_tensor(
                                ct_tmp[:, qh * 512 : (qh + 1) * 512],
                                pv[:],
                                bc_sb[0:64, qh * 512 : (qh + 1) * 512],
                                op=MULT,
                            )
                        nc.sync.dma_start(ct_sb[64:128, hc, :], ct_tmp[:])

            # ---- Phase 3: output projection (partial; host sums core pairs) ----
            with (
                tc.tile_pool(name="ops", bufs=2, space="PSUM") as oppsum,
                tc.tile_pool(name="osb", bufs=2) as outpool,
            ):
                for ch in range(8):
                    o_sb = outpool.tile([128, 1024], F32, tag="osb")
                    for sh in range(2):
                        o_ps = oppsum.tile([128, 512], F32, tag="ops")
                        for ce in range(4):
                            nc.tensor.matmul(
                                o_ps[:],
                                wo_sb[:, ce, ch * 128 : (ch + 1) * 128],
                                ct_sb[:, ce, sh * 512 : (sh + 1) * 512],
                                start=(ce == 0),
                                stop=(ce == 3),
                            )
                        nc.vector.tensor_scalar_add(
                            o_sb[:, sh * 512 : (sh + 1) * 512],
                            o_ps[:],
                            bo_sb[:, ch : ch + 1],
                        )
                    nc.sync.dma_start(
                        outp_d.ap()[ch * 128 : (ch + 1) * 128, :], o_sb[:]
                    )

    nc.compile()
    _CACHE["nc"] = nc
    return nc


def kernel(**inputs):
    query = np.asarray(inputs["query"], np.float32)
    key = np.asarray(inputs["key"], np.float32)
    value = np.asarray(inputs["value"], np.float32)
    Wq, bq = np.asarray(inputs["Wq"], np.float32), np.asarray(inputs["bq"], np.float32)
    Wk, bk = np.asarray(inputs["Wk"], np.float32), np.asarray(inputs["bk"], np.float32)
    Wv, bv = np.asarray(inputs["Wv"], np.float32), np.asarray(inputs["bv"], np.float32)
    Wo, bo = np.asarray(inputs["Wo"], np.float32), np.asarray(inputs["bo"], np.float32)

    nc = _build()

    ones = np.ones((1, 128), np.float32)
    in_maps = []
    for c in range(NCORES):
        b, g = divmod(c, 2)
        cols = slice(g * ESL, (g + 1) * ESL)
        # bv folds through the (linear) output projection: W @ (V + bv) adds
        # Wo_c^T @ bv_c per core; bo itself is added by the even core only.
        bo_eff = Wo.T[cols, :].T @ bv[cols]
        if g == 0:
            bo_eff = bo_eff + bo
        in_maps.append(
            {
                "xtq": np.ascontiguousarray(query[b].T),
                "xtk": np.ascontiguousarray(key[b].T),
                "xtv": np.ascontiguousarray(value[b].T),
                "wqt": np.ascontiguousarray(Wq.T[:, cols]),
                "wkt": np.ascontiguousarray(Wk.T[:, cols]),
                "wvt": np.ascontiguousarray(Wv.T[:, cols]),
                "wot": np.ascontiguousarray(Wo.T[cols, :]),
                "bq": np.ascontiguousarray(bq[cols].reshape(4, 128).T),
                "bk": np.ascontiguousarray(bk[cols].reshape(4, 128).T),
                "bo": np.ascontiguousarray(bo_eff.reshape(8, 128).T),
                "ones": ones,
            }
        )

    res = run_bass_kernel_spmd(nc, in_maps, list(range(NCORES)))

    out = np.empty((B, S, EMB), np.float32)
    wts = np.empty((B, HEADS, S, S), np.float32)
    for c in range(NCORES):
        b, g = divmod(c, 2)
        wts[b, g * HPC : (g + 1) * HPC] = res.results[c]["wts"]
    for b in range(B):
        out[b] = (res.results[2 * b]["outp"] + res.results[2 * b + 1]["outp"]).T
    return out, wts


# revision 6
# speedup vs baseline: 2.7012x; 2.7012x over previous
"""Multi-head attention Trainium2 kernel (B=4, S=1024, EMB=1024, 16 heads).

Sharding: 8 cores = 4 batches x 2 head-groups. Core c handles batch c//2 and
heads [8*(c%2), 8*(c%2)+8) -- tensor-parallel over heads within a batch.
Each core computes its Q/K/V projections (512 of 1024 e_out columns), full
attention for its 8 heads, and a partial output projection; the two cores
sharing a batch have their partials summed on the host.

Device layouts (per core):
  QT/KT: [e_out, s] transposed projections as SBUF [128p, 4chunk, 1024s]
         (e_out local = chunk*128 + p; head h at chunk h//2, partitions
         64*(h%2)..+64)
  V:     natural [s, e_out] as SBUF [128p, 8st, 512]
  Scores are computed twice per head: S=[q,k] (softmax rows on partitions,
  normalized weights output) and S^T=[k,q] (feeds P@V with k on partitions).
  Softmax denominators come free from activation(Exp, accum_out=...).
  All matmuls run in float32r (~1.5e-4 rel err, full PE rate at N=512).
"""

import numpy as np

import concourse.bacc as bacc
import concourse.mybir as mybir
import concourse.tile as tile
from concourse.bass_utils import run_bass_kernel_spmd

B, S, EMB, HEADS, HD = 4, 1024, 1024, 16, 64
SCALE = HD**-0.5
NCORES = 8
HPC = HEADS // 2  # heads per core
ESL = HPC * HD  # e_out slice per core (512)
F32 = mybir.dt.float32
F32R = mybir.dt.float32r
EXP = mybir.ActivationFunctionType.Exp
MULT = mybir.AluOpType.mult

_CACHE = {}


def _build():
    if "nc" in _CACHE:
        return _CACHE["nc"]

    nc = bacc.Bacc("TRN2", target_bir_lowering=False, debug=False, num_devices=NCORES)

    xtq = nc.dram_tensor("xtq", [EMB, S], F32R, kind="ExternalInput")
    xtk = nc.dram_tensor("xtk", [EMB, S], F32R, kind="ExternalInput")
    xtv = nc.dram_tensor("xtv", [EMB, S], F32R, kind="ExternalInput")
    wqt = nc.dram_tensor("wqt", [EMB, ESL], F32R, kind="ExternalInput")
    wkt = nc.dram_tensor("wkt", [EMB, ESL], F32R, kind="ExternalInput")
    wvt = nc.dram_tensor("wvt", [EMB, ESL], F32R, kind="ExternalInput")
    wot = nc.dram_tensor("wot", [ESL, EMB], F32R, kind="ExternalInput")
    bq_d = nc.dram_tensor("bq", [128, 4], F32, kind="ExternalInput")
    bk_d = nc.dram_tensor("bk", [128, 4], F32, kind="ExternalInput")
    bo_d = nc.dram_tensor("bo", [128, 8], F32, kind="ExternalInput")
    ones_d = nc.dram_tensor("ones", [1, 128], F32R, kind="ExternalInput")
    wts_d = nc.dram_tensor("wts", [HPC, S, S], F32, kind="ExternalOutput")
    outp_d = nc.dram_tensor("outp", [EMB, S], F32, kind="ExternalOutput")

    with tile.TileContext(nc) as tc, nc.allow_low_precision(
        reason="float32r tiles feed full-rate PE matmuls; accumulation stays fp32"
    ):
        with (
            tc.tile_pool(name="const", bufs=1) as cpool,
            tc.tile_pool(name="qkv", bufs=1) as qkvpool,
            tc.tile_pool(name="wt", bufs=2) as wtpool,
        ):
            bq_sb = cpool.tile([128, 4], F32)
            bk_sb = cpool.tile([128, 4], F32)
            bo_sb = cpool.tile([128, 8], F32)
            ones_sb = cpool.tile([1, 128], F32R)
            nc.sync.dma_start(bq_sb[:], bq_d.ap())
            nc.sync.dma_start(bk_sb[:], bk_d.ap())
            nc.sync.dma_start(bo_sb[:], bo_d.ap())
            nc.sync.dma_start(ones_sb[:], ones_d.ap())

            qt_sb = qkvpool.tile([128, 4, S], F32R)
            kt_sb = qkvpool.tile([128, 4, S], F32R)
            v_sb = qkvpool.tile([128, 8, ESL], F32R)

            # ---- Phase 1: projections ----
            with (
                tc.tile_pool(name="xt", bufs=3) as xtpool,
                tc.tile_pool(name="pjps", bufs=8, space="PSUM") as pjps,
            ):
                for pname, xdram, wdram, bias, dst in (
                    ("q", xtq, wqt, bq_sb, qt_sb),
                    ("k", xtk, wkt, bk_sb, kt_sb),
                    ("v", xtv, wvt, None, v_sb),
                ):
                    w_sb = wtpool.tile([128, 8, ESL], F32R, tag="wt", name=f"w_{pname}")
                    nc.sync.dma_start(
                        w_sb[:], wdram.ap().rearrange("(kt p) n -> p kt n", p=128)
                    )
                    ps = [
                        pjps.tile([128, 512], F32, tag="pjps", name=f"pj_{pname}_{i}")
                        for i in range(8)
                    ]
                    for kt in range(8):
                        x_t = xtpool.tile([128, S], F32R, tag="xt", name=f"x_{pname}_{kt}")
                        nc.sync.dma_start(
                            x_t[:], xdram.ap()[kt * 128 : (kt + 1) * 128, :]
                        )
                        if pname == "v":
                            # V natural [s, e_out]: lhsT = X^T tile cols
                            for st in range(8):
                                nc.tensor.matmul(
                                    ps[st][:],
                                    x_t[:, st * 128 : (st + 1) * 128],
                                    w_sb[:, kt, :],
                                    start=(kt == 0),
                                    stop=(kt == 7),
                                )
                        else:
                            # Q^T/K^T: lhsT = W^T tile cols, rhs = X^T
                            for ch in range(4):
                                for sh in range(2):
                                    nc.tensor.matmul(
                                        ps[ch * 2 + sh][:],
                                        w_sb[:, kt, ch * 128 : (ch + 1) * 128],
                                        x_t[:, sh * 512 : (sh + 1) * 512],
                                        start=(kt == 0),
                                        stop=(kt == 7),
                                    )
                    if pname == "v":
                        for st in range(8):
                            nc.vector.tensor_copy(v_sb[:, st, :], ps[st][:])
                    else:
                        for ch in range(4):
                            for sh in range(2):
                                nc.vector.tensor_scalar_add(
                                    dst[:, ch, sh * 512 : (sh + 1) * 512],
                                    ps[ch * 2 + sh][:],
                                    bias[:, ch : ch + 1],
                                )

            # ---- Phase 2: attention per head ----
            ct_sb = qkvpool.tile([128, 4, S], F32R)
            wo_sb = wtpool.tile([128, 4, EMB], F32R, tag="wt")
            nc.sync.dma_start(
                wo_sb[:], wot.ap().rearrange("(ce p) n -> p ce n", p=128)
            )

            with (
                tc.tile_pool(name="sps", bufs=1, space="PSUM") as spsum,
                tc.tile_pool(name="stps", bufs=1, space="PSUM") as stpsum,
                tc.tile_pool(name="pvps", bufs=2, space="PSUM") as pvpsum,
                tc.tile_pool(name="bcps", bufs=1, space="PSUM") as bcpsum,
                tc.tile_pool(name="enat", bufs=3) as epool,
                tc.tile_pool(name="wsb", bufs=3) as wpool,
                tc.tile_pool(name="et", bufs=3) as etpool,
                tc.tile_pool(name="small", bufs=2) as smallpool,
                tc.tile_pool(name="bc", bufs=2) as bcpool,
            ):
                for h in range(HPC):
                    hb = 64 * (h % 2)
                    hc = h // 2
                    tp = (hb, 0)
                    acc_h = smallpool.tile([128, 8], F32, tag="acc")

                    # S = Q K^T rows-on-partitions; exp + row sums; weights out
                    for qt in range(8):
                        s_ps = spsum.tile([128, 1024], F32, tag="sps")
                        for kh in range(2):
                            nc.tensor.matmul(
                                s_ps[:, kh * 512 : (kh + 1) * 512],
                                qt_sb[hb : hb + 64, hc, qt * 128 : (qt + 1) * 128],
                                kt_sb[hb : hb + 64, hc, kh * 512 : (kh + 1) * 512],
                                start=True,
                                stop=True,
                                tile_position=tp,
                            )
                        e_t = epool.tile([128, 1024], F32, tag="enat")
                        nc.scalar.activation(
                            e_t[:],
                            s_ps[:],
                            EXP,
                            scale=SCALE,
                            accum_out=acc_h[:, qt : qt + 1],
                        )
                        rq = smallpool.tile([128, 1], F32, tag="rq")
                        nc.vector.reciprocal(rq[:], acc_h[:, qt : qt + 1])
                        w_t = wpool.tile([128, 1024], F32, tag="wsb")
                        nc.vector.tensor_scalar_mul(w_t[:], e_t[:], rq[:])
                        nc.sync.dma_start(
                            wts_d.ap()[h, qt * 128 : (qt + 1) * 128, :], w_t[:]
                        )

                    # reciprocal row-sums broadcast tile [128, q] for PV scaling
                    sumsf = smallpool.tile([1, 1024], F32, tag="sumsf")
                    for qt in range(8):
                        nc.sync.dma_start(
                            sumsf[0:1, qt * 128 : (qt + 1) * 128],
                            acc_h[:, qt : qt + 1],
                        )
                    recipf = smallpool.tile([1, 1024], F32R, tag="recipf")
                    nc.vector.reciprocal(recipf[:], sumsf[:])
                    bc_sb = bcpool.tile([128, 1024], F32R, tag="bc")
                    for qh in range(2):
                        bc_ps = bcpsum.tile([128, 512], F32, tag="bcps")
                        nc.tensor.matmul(
                            bc_ps[:],
                            ones_sb[0:1, :],
                            recipf[0:1, qh * 512 : (qh + 1) * 512],
                            start=True,
                            stop=True,
                        )
                        nc.vector.tensor_copy(
                            bc_sb[:, qh * 512 : (qh + 1) * 512], bc_ps[:]
                        )

                    # S^T (k on partitions) -> exp -> P@V accumulation
                    pv0 = pvpsum.tile([64, 512], F32, tag="pv")
                    pv1 = pvpsum.tile([64, 512], F32, tag="pv")
                    for kt in range(8):
                        st_ps = stpsum.tile([128, 1024], F32, tag="stps")
                        for qh in range(2):
                            nc.tensor.matmul(
                                st_ps[:, qh * 512 : (qh + 1) * 512],
                                kt_sb[hb : hb + 64, hc, kt * 128 : (kt + 1) * 128],
                                qt_sb[hb : hb + 64, hc, qh * 512 : (qh + 1) * 512],
                                start=True,
                                stop=True,
                                tile_position=tp,
                            )
                        et_t = etpool.tile([128, 1024], F32R, tag="et")
                        nc.scalar.activation(et_t[:], st_ps[:], EXP, scale=SCALE)
                        for qh, pv in ((0, pv0), (1, pv1)):
                            nc.tensor.matmul(
                                pv[:],
                                v_sb[:, kt, h * 64 : (h + 1) * 64],
                                et_t[:, qh * 512 : (qh + 1) * 512],
                                start=(kt == 0),
                                stop=(kt == 7),
                            )
                    # matmul dst must start at partition 0; odd heads hop to
                    # partitions 64-127 of ct_sb via a small SBUF->SBUF DMA
                    if h % 2 == 0:
                        for qh, pv in ((0, pv0), (1, pv1)):
                            nc.vector.tensor_tensor(
                                ct_sb[0:64, hc, qh * 512 : (qh + 1) * 512],
                                pv[:],
                                bc_sb[0:64, qh * 512 : (qh + 1) * 512],
                                op=MULT,
                            )
                    else:
                        ct_tmp = bcpool.tile([64, 1024], F32R, tag="cttmp")
                        for qh, pv in ((0, pv0), (1, pv1)):
                            nc.vector.tensor_tensor(
                                ct_tmp[:, qh * 512 : (qh + 1) * 512],
                                pv[:],
                                bc_sb[0:64, qh * 512 : (qh + 1) * 512],
                                op=MULT,
                            )
                        nc.sync.dma_start(ct_sb[64:128, hc, :], ct_tmp[:])

            # ---- Phase 3: output projection (partial; host sums core pairs) ----
            with (
                tc.tile_pool(name="ops", bufs=2, space="PSUM") as oppsum,
                tc.tile_pool(name="osb", bufs=2) as outpool,
            ):
                for ch in range(8):
                    o_sb = outpool.tile([128, 1024], F32, tag="osb")
                    for sh in range(2):
                        o_ps = oppsum.tile([128, 512], F32, tag="ops")
                        for ce in range(4):
                            nc.tensor.matmul(
                                o_ps[:],
                                wo_sb[:, ce, ch * 128 : (ch + 1) * 128],
                                ct_sb[:, ce, sh * 512 : (sh + 1) * 512],
                                start=(ce == 0),
                                stop=(ce == 3),
                            )
                        nc.vector.tensor_scalar_add(
                            o_sb[:, sh * 512 : (sh + 1) * 512],
                            o_ps[:],
                            bo_sb[:, ch : ch + 1],
                        )
                    nc.sync.dma_start(
                        outp_d.ap()[ch * 128 : (ch + 1) * 128, :], o_sb[:]
                    )

    nc.compile()
    _CACHE["nc"] = nc
    return nc


def kernel(**inputs):
    query = np.asarray(inputs["query"], np.float32)
    key = np.asarray(inputs["key"], np.float32)
    value = np.asarray(inputs["value"], np.float32)
    Wq, bq = np.asarray(inputs["Wq"], np.float32), np.asarray(inputs["bq"], np.float32)
    Wk, bk = np.asarray(inputs["Wk"], np.float32), np.asarray(inputs["bk"], np.float32)
    Wv, bv = np.asarray(inputs["Wv"], np.float32), np.asarray(inputs["bv"], np.float32)
    Wo, bo = np.asarray(inputs["Wo"], np.float32), np.asarray(inputs["bo"], np.float32)

    nc = _build()

    ones = np.ones((1, 128), np.float32)
    in_maps = []
    for c in range(NCORES):
        b, g = divmod(c, 2)
        cols = slice(g * ESL, (g + 1) * ESL)
        # bv folds through the (linear) output projection: W @ (V + bv) adds
        # Wo_c^T @ bv_c per core; bo itself is added by the even core only.
        bo_eff = Wo.T[cols, :].T @ bv[cols]
        if g == 0:
            bo_eff = bo_eff + bo
        in_maps.append(
            {
                "xtq": np.ascontiguousarray(query[b].T),
                "xtk": np.ascontiguousarray(key[b].T),
                "xtv": np.ascontiguousarray(value[b].T),
                "wqt": np.ascontiguousarray(Wq.T[:, cols]),
                "wkt": np.ascontiguousarray(Wk.T[:, cols]),
                "wvt": np.ascontiguousarray(Wv.T[:, cols]),
                "wot": np.ascontiguousarray(Wo.T[cols, :]),
                "bq": np.ascontiguousarray(bq[cols].reshape(4, 128).T),
                "bk": np.ascontiguousarray(bk[cols].reshape(4, 128).T),
                "bo": np.ascontiguousarray(bo_eff.reshape(8, 128).T),
                "ones": ones,
            }
        )

    res = run_bass_kernel_spmd(nc, in_maps, list(range(NCORES)))

    out = np.empty((B, S, EMB), np.float32)
    wts = np.empty((B, HEADS, S, S), np.float32)
    for c in range(NCORES):
        b, g = divmod(c, 2)
        wts[b, g * HPC : (g + 1) * HPC] = res.results[c]["wts"]
    for b in range(B):
        out[b] = (res.results[2 * b]["outp"] + res.results[2 * b + 1]["outp"]).T
    return out, wts
